# revision 5
# baseline (speedup 1.0000x reference)
"""Llama attention (B=2, S=2048, H=4096, 32 heads / 8 KV heads) on 8
Trainium2 NeuronCores.

Sharding: query-token parallel. Core c handles batch c//4, query rows
[512*(c%4), 512*(c%4)+512). Each core computes the full K/V projection for
its batch (replicated within its 4-core batch group), attention for its
512 query rows against all 2048 keys (causality via a 0/1 mask supplied as
data), and the o_proj for its rows. No collectives; the host concatenates
the 8 per-core row slices into the full output.

All matmuls run in fp16 with fp32 PSUM accumulation. The host pre-packs
every operand into the exact SBUF tile layout (hidden-on-partitions,
128-partition tiles) so device DMAs are large contiguous reads.
"""
import sys
import os
sys.path.insert(0, '/opt/trn_rl_repo')
import numpy as np

FULL_CFG = dict(H=4096, NH=32, NKV=8, HD=128, B=2, S=2048, NC=8)


def derive(cfg):
    d = dict(cfg)
    d['hk'] = cfg['H'] // 128            # hidden 128-tiles
    d['Q'] = cfg['S'] // (cfg['NC'] // cfg['B'])   # query rows per core
    d['QC'] = d['Q'] // 128              # query 128-chunks
    d['KT'] = cfg['S'] // 128            # key 128-tiles
    d['NP'] = cfg['S'] // 512            # 512-token panels
    d['VD'] = cfg['NKV'] * 128           # total kv dim
    d['VW'] = min(512, d['VD'])          # vd panel width
    d['VDP'] = d['VD'] // d['VW']
    d['FT'] = cfg['H'] // 128            # o_proj f tiles (= hk)
    d['OW'] = min(512, cfg['H'])         # o_proj out panel width
    d['OD'] = cfg['H'] // d['OW']
    return d


def build_kernel(cfg, repeat=1):
    import concourse.bacc as bacc
    import concourse.tile as tile
    import concourse.mybir as mybir

    d = derive(cfg)
    H, S, Q, NH, NKV = cfg['H'], cfg['S'], d['Q'], cfg['NH'], cfg['NKV']
    hk, QC, KT, NP = d['hk'], d['QC'], d['KT'], d['NP']
    VW, VDP, FT, OW, OD = d['VW'], d['VDP'], d['FT'], d['OW'], d['OD']
    f16, f32 = mybir.dt.float16, mybir.dt.float32
    EXP = mybir.ActivationFunctionType.Exp
    SCALE = 1.0 / float(np.sqrt(cfg['HD']))
    EBIAS = -5.0

    nc = bacc.Bacc(target_bir_lowering=False)
    with tile.TileContext(nc) as tc:
        with tc.tile_pool(name="dram", bufs=1, space="DRAM") as dram:
            xt_d = dram.tile([128, hk, S], f16, kind="ExternalInput")
            wq_d = dram.tile([128, NH, hk, 128], f16, kind="ExternalInput")
            wk_d = dram.tile([128, NKV, hk, 128], f16, kind="ExternalInput")
            wv_d = dram.tile([128, VDP, hk, VW], f16, kind="ExternalInput")
            wo_d = dram.tile([128, OD, FT, OW], f16, kind="ExternalInput")
            ck_d = dram.tile([128, S], f16, kind="ExternalInput")
            sk_d = dram.tile([128, S], f16, kind="ExternalInput")
            mk_d = dram.tile([128, KT, Q], f16, kind="ExternalInput")
            out_d = dram.tile([Q, H], f32, kind="ExternalOutput")
            names = dict(xt=xt_d.name, wq=wq_d.name, wk=wk_d.name,
                         wv=wv_d.name, wo=wo_d.name, ck=ck_d.name,
                         sk=sk_d.name, mk=mk_d.name, out=out_d.name)

            with tc.tile_pool(name="const", bufs=1) as cpool, \
                 tc.tile_pool(name="res", bufs=1) as res, \
                 tc.tile_pool(name="qa", bufs=NH + 2) as qa, \
                 tc.tile_pool(name="rope", bufs=2) as rope, \
                 tc.tile_pool(name="ptile", bufs=3) as ptile, \
                 tc.tile_pool(name="ps2", bufs=2, space="PSUM") as ps2, \
                 tc.tile_pool(name="psd", bufs=2, space="PSUM") as psd:

                bias_t = cpool.tile([128, 1], f32, name="bias_t")
                nc.vector.memset(bias_t[:], EBIAS)
                ones_c = cpool.tile([128, 1], f16, name="ones_c")
                nc.vector.memset(ones_c[:], 1.0)

                ck_s = res.tile([128, S], f16, name="ck_s")
                sk_s = res.tile([128, S], f16, name="sk_s")
                nc.sync.dma_start(ck_s[:], ck_d[:])
                nc.sync.dma_start(sk_s[:], sk_d[:])

                kt_sb = res.tile([128, NKV * S], f16, name="kt_sb")
                v_sb = res.tile([128, KT * NKV * 128], f16, name="v_sb")

                def rope_evict(psum, out_slice, tab_lo, tab_hi):
                    # out = base*cos + shift64(base)*sin' over token cols
                    # [tab_lo, tab_hi) of the rope tables
                    n = tab_hi - tab_lo
                    base = rope.tile([128, n], f32, tag="base", name="base")
                    shf = rope.tile([128, n], f32, tag="shf", name="shf")
                    m1 = rope.tile([128, n], f32, tag="m1", name="m1")
                    nc.scalar.copy(base[:], psum[:])
                    nc.sync.dma_start(shf[0:64, :], base[64:128, :])
                    nc.sync.dma_start(shf[64:128, :], base[0:64, :])
                    nc.vector.tensor_mul(m1[:], shf[:],
                                         sk_s[:, tab_lo:tab_hi])
                    nc.vector.tensor_mul(base[:], base[:],
                                         ck_s[:, tab_lo:tab_hi])
                    nc.vector.tensor_add(out_slice, m1[:], base[:])

                for rep in range(repeat):
                    # ---- phase A: q-proj + RoPE -> qt tiles ----
                    qt_tiles = []
                    with tc.tile_pool(name=f"stA{rep}", bufs=2) as stA, \
                         tc.tile_pool(name=f"xtA{rep}", bufs=1) as xtA:
                        xt0 = xtA.tile([128, hk * Q], f16, tag="xt",
                                       name=f"xt0_{rep}")
                        nc.sync.dma_start(
                            xt0[:].rearrange("p (a b) -> p a b", a=hk),
                            xt_d[:, :, 0:Q])
                        for qd in range(NH):
                            wq_t = stA.tile([128, hk * 128], f16, tag="wq",
                                            name=f"wq{qd}")
                            nc.sync.dma_start(
                                wq_t[:].rearrange("p (a b) -> p a b", a=hk),
                                wq_d[:, qd])
                            pq = ps2.tile([128, Q], f32, tag="pp",
                                          name=f"pq{qd}")
                            for k in range(hk):
                                nc.tensor.matmul(
                                    pq[:], wq_t[:, k * 128:(k + 1) * 128],
                                    xt0[:, k * Q:k * Q + Q],
                                    start=(k == 0), stop=(k == hk - 1))
                            qt = qa.tile([128, Q], f16, tag="qa",
                                         name=f"qt{qd}")
                            rope_evict(pq[:], qt[:], 0, Q)
                            qt_tiles.append(qt)

                    # ---- phase B: V-proj (token-major) ----
                    with tc.tile_pool(name=f"stB{rep}", bufs=1) as stB, \
                         tc.tile_pool(name=f"xtB{rep}", bufs=1) as xtB:
                        for vdp in range(VDP):
                            wv_t = stB.tile([128, hk * VW], f16, tag="wv",
                                            name=f"wv{vdp}")
                            nc.sync.dma_start(
                                wv_t[:].rearrange("p (a b) -> p a b", a=hk),
                                wv_d[:, vdp])
                            for pan in range(NP):
                                xt1 = xtB.tile([128, hk * 512], f16,
                                               tag="xt",
                                               name=f"xtv{vdp}_{pan}")
                                nc.sync.dma_start(
                                    xt1[:].rearrange(
                                        "p (a b) -> p a b", a=hk),
                                    xt_d[:, :, pan * 512:(pan + 1) * 512])
                                for tt in range(4):
                                    pv = ps2.tile([128, VW], f32, tag="pp",
                                                  name=f"pv{vdp}{pan}{tt}")
                                    for k in range(hk):
                                        nc.tensor.matmul(
                                            pv[:],
                                            xt1[:, k * 512 + tt * 128:
                                                k * 512 + tt * 128 + 128],
                                            wv_t[:, k * VW:(k + 1) * VW],
                                            start=(k == 0),
                                            stop=(k == hk - 1))
                                    col = ((pan * 4 + tt) * (NKV * 128)
                                           + vdp * VW)
                                    nc.scalar.copy(
                                        v_sb[:, col:col + VW], pv[:])

                    # ---- phase C: K-proj + RoPE (f-major) ----
                    with tc.tile_pool(name=f"stC{rep}", bufs=2) as stC, \
                         tc.tile_pool(name=f"xtC{rep}", bufs=1) as xtC:
                        for pan in range(NP):
                            xt2 = xtC.tile([128, hk * 512], f16, tag="xt",
                                           name=f"xtk{pan}")
                            nc.sync.dma_start(
                                xt2[:].rearrange("p (a b) -> p a b", a=hk),
                                xt_d[:, :, pan * 512:(pan + 1) * 512])
                            for kd in range(NKV):
                                wk_t = stC.tile([128, hk * 128], f16,
                                                tag="wk",
                                                name=f"wk{pan}_{kd}")
                                nc.sync.dma_start(
                                    wk_t[:].rearrange(
                                        "p (a b) -> p a b", a=hk),
                                    wk_d[:, kd])
                                pk = ps2.tile([128, 512], f32, tag="pp",
                                              name=f"pk{pan}_{kd}")
                                for k in range(hk):
                                    nc.tensor.matmul(
                                        pk[:],
                                        wk_t[:, k * 128:(k + 1) * 128],
                                        xt2[:, k * 512:(k + 1) * 512],
                                        start=(k == 0), stop=(k == hk - 1))
                                rope_evict(
                                    pk[:],
                                    kt_sb[:, kd * S + pan * 512:
                                          kd * S + pan * 512 + 512],
                                    pan * 512, (pan + 1) * 512)

                    # ---- phase D: attention ----
                    at_tiles = []
                    with tc.tile_pool(name=f"mkp{rep}", bufs=1) as mkp:
                        mk_s = mkp.tile([128, KT * Q], f16,
                                        name=f"mk_s{rep}")
                        nc.sync.dma_start(
                            mk_s[:].rearrange("p (a b) -> p a b", a=KT),
                            mk_d[:])
                        for h in range(NH):
                            kv = h // (NH // NKV)
                            ppv = ps2.tile([128, Q], f32, tag="pv",
                                           name=f"ppv{h}")
                            pden = psd.tile([1, Q], f32, tag="den",
                                            name=f"pden{h}")
                            for kt in range(KT):
                                pst = ps2.tile([128, Q], f32, tag="s",
                                               name=f"pst{h}_{kt}")
                                nc.tensor.matmul(
                                    pst[:],
                                    kt_sb[:, kv * S + kt * 128:
                                          kv * S + kt * 128 + 128],
                                    qt_tiles[h][:],
                                    start=True, stop=True)
                                pt = ptile.tile([128, Q], f16, tag="pt",
                                                name=f"pt{h}_{kt}")
                                nc.scalar.activation(
                                    pt[:], pst[:], EXP,
                                    bias=bias_t[:], scale=SCALE)
                                nc.vector.tensor_mul(
                                    pt[:], pt[:],
                                    mk_s[:, kt * Q:(kt + 1) * Q])
                                nc.tensor.matmul(
                                    ppv[:],
                                    v_sb[:, kt * (NKV * 128) + kv * 128:
                                         kt * (NKV * 128) + kv * 128 + 128],
                                    pt[:],
                                    start=(kt == 0), stop=(kt == KT - 1))
                                nc.tensor.matmul(
                                    pden[:], ones_c[:], pt[:],
                                    start=(kt == 0), stop=(kt == KT - 1))
                            den_r = rope.tile([1, Q], f32, tag="den_r",
                                              name=f"den_r{h}")
                            nc.vector.reciprocal(den_r[:], pden[:])
                            bc = rope.tile([128, Q], f32, tag="bc",
                                           name=f"bc{h}")
                            nc.gpsimd.partition_broadcast(bc[:], den_r[:])
                            at = qa.tile([128, Q], f16, tag="qa",
                                         name=f"at{h}")
                            nc.vector.tensor_mul(at[:], ppv[:], bc[:])
                            at_tiles.append(at)

                    # ---- phase E: o-proj ----
                    with tc.tile_pool(name=f"stE{rep}", bufs=2) as stE:
                        for od in range(OD):
                            wo_t = stE.tile([128, FT * OW], f16, tag="wo",
                                            name=f"wo{od}")
                            nc.sync.dma_start(
                                wo_t[:].rearrange("p (a b) -> p a b", a=FT),
                                wo_d[:, od])
                            for qc in range(QC):
                                po = ps2.tile([128, OW], f32, tag="pp",
                                              name=f"po{od}_{qc}")
                                for ft in range(FT):
                                    fh = ft * 128 // cfg['HD']
                                    nc.tensor.matmul(
                                        po[:],
                                        at_tiles[fh][:, qc * 128:
                                                      qc * 128 + 128],
                                        wo_t[:, ft * OW:(ft + 1) * OW],
                                        start=(ft == 0),
                                        stop=(ft == FT - 1))
                                stg = ptile.tile([128, OW], f32, tag="stg",
                                                 name=f"stg{od}_{qc}")
                                nc.scalar.copy(stg[:], po[:])
                                nc.sync.dma_start(
                                    out_d[qc * 128:(qc + 1) * 128,
                                          od * OW:(od + 1) * OW], stg[:])
    nc.compile()
    return nc, names


def prep_core_inputs(cfg, names, hidden, position_ids, Wq, Wk, Wv, Wo, core):
    d = derive(cfg)
    H, S, Q, NH, NKV, HD = (cfg['H'], cfg['S'], d['Q'], cfg['NH'],
                            cfg['NKV'], cfg['HD'])
    hk, KT = d['hk'], d['KT']
    VW, VDP, FT, OW, OD = d['VW'], d['VDP'], d['FT'], d['OW'], d['OD']
    npb = cfg['NC'] // cfg['B']
    b, p = core // npb, core % npb
    f16 = np.float16

    X = np.roll(np.asarray(hidden[b]), -p * Q, axis=0)      # [S, H]
    xt = np.ascontiguousarray(
        X.T.reshape(hk, 128, S).transpose(1, 0, 2)).astype(f16)

    wq = np.ascontiguousarray(
        np.asarray(Wq).reshape(NH, 128, hk, 128)
        .transpose(3, 0, 2, 1)).astype(f16)
    wk = np.ascontiguousarray(
        np.asarray(Wk).reshape(NKV, 128, hk, 128)
        .transpose(3, 0, 2, 1)).astype(f16)
    wv = np.ascontiguousarray(
        np.asarray(Wv).reshape(VDP, VW, hk, 128)
        .transpose(3, 0, 2, 1)).astype(f16)
    wo = np.ascontiguousarray(
        np.asarray(Wo).reshape(OD, OW, FT, 128)
        .transpose(3, 0, 2, 1)).astype(f16)

    pos = np.roll(np.asarray(position_ids[b]).astype(np.float64), -p * Q)
    inv_freq = 1.0 / (10000.0 ** (np.arange(0, HD, 2) / HD))
    ang = pos[:, None] * inv_freq[None, :]                  # [S, HD/2]
    ang2 = np.concatenate([ang, ang], axis=1).T             # [HD, S]
    sk = np.sin(ang2).astype(np.float32)
    ck = np.cos(ang2).astype(np.float32)
    sk[0:HD // 2, :] *= -1.0

    gq = p * Q + np.arange(Q)
    gk = (np.arange(S) + p * Q) % S
    mask = (gk[:, None] <= gq[None, :]).astype(f16)         # [S, Q]
    mk = np.ascontiguousarray(
        mask.reshape(KT, 128, Q).transpose(1, 0, 2))

    return {
        names['xt']: xt,
        names['wq']: wq, names['wk']: wk, names['wv']: wv, names['wo']: wo,
        names['ck']: ck.astype(f16), names['sk']: sk.astype(f16),
        names['mk']: mk,
    }


# ---------------------------------------------------------------------------
# PJRT runner (embedded so kernel.py is self-contained)
# ---------------------------------------------------------------------------
class BassRunner:
    def __init__(self, nc, n_cores):
        import jax
        import concourse.mybir as mybir
        from concourse.bass2jax import (
            _bass_exec_p, install_neuronx_cc_hook, partition_id_tensor)
        from jax.sharding import Mesh, PartitionSpec
        from jax.experimental.shard_map import shard_map
        self.jax = jax
        install_neuronx_cc_hook()
        self.nc = nc
        self.n_cores = n_cores

        partition_name = (nc.partition_id_tensor.name
                          if nc.partition_id_tensor else None)
        dbg_name = nc.dbg_addr.name if nc.dbg_addr is not None else None
        in_names, out_names, out_avals, zero_outs = [], [], [], []
        for alloc in nc.m.functions[0].allocations:
            if not isinstance(alloc, mybir.MemoryLocationSet):
                continue
            name = alloc.memorylocations[0].name
            if alloc.kind == "ExternalInput":
                if name not in (partition_name, dbg_name):
                    in_names.append(name)
            elif alloc.kind == "ExternalOutput":
                shape = tuple(alloc.tensor_shape)
                dtype = mybir.dt.np(alloc.dtype)
                out_names.append(name)
                out_avals.append(jax.core.ShapedArray(shape, dtype))
                zero_outs.append(np.zeros(shape, dtype))
        self.in_names, self.out_names = in_names, out_names
        self.out_avals, self.zero_outs = out_avals, zero_outs
        n_params = len(in_names)
        self.n_params = n_params

        all_in_names = list(in_names) + list(out_names)
        if dbg_name is not None:
            all_in_names.append(dbg_name)
        if partition_name is not None:
            all_in_names.append(partition_name)
        has_dbg = dbg_name is not None

        def _body(*args):
            operands = list(args)
            if has_dbg:
                operands.append(jax.numpy.zeros((1, 2), jax.numpy.uint32))
            if partition_name is not None:
                operands.append(partition_id_tensor())
            outs = _bass_exec_p.bind(
                *operands,
                out_avals=tuple(out_avals),
                in_names=tuple(all_in_names),
                out_names=tuple(out_names),
                lowering_input_output_aliases=(),
                sim_require_finite=False,
                sim_require_nnan=False,
                nc=nc,
            )
            return tuple(outs)

        devices = [d for d in jax.devices() if d.platform != 'cpu'][:n_cores]
        assert len(devices) == n_cores, \
            f"need {n_cores} devices, have {len(jax.devices())}"
        self._sharding = None
        if n_cores == 1:
            self._device = devices[0]
            self._fn = jax.jit(_body, keep_unused=True)
        else:
            mesh = Mesh(np.asarray(devices), ("core",))
            self._sharding = jax.sharding.NamedSharding(
                mesh, PartitionSpec("core"))
            in_specs = (PartitionSpec("core"),) * (n_params + len(out_names))
            out_specs = (PartitionSpec("core"),) * len(out_names)
            self._fn = jax.jit(
                shard_map(_body, mesh=mesh, in_specs=in_specs,
                          out_specs=out_specs, check_rep=False),
                keep_unused=True)
        self._device_args = None

    def stage(self, in_maps):
        assert len(in_maps) == self.n_cores
        n = self.n_cores
        if n == 1:
            args = [np.asarray(in_maps[0][nm]) for nm in self.in_names]
            args += list(self.zero_outs)
        else:
            args = [np.concatenate([np.asarray(in_maps[c][nm])
                                    for c in range(n)], axis=0)
                    for nm in self.in_names]
            args += [np.zeros((n * z.shape[0], *z.shape[1:]), z.dtype)
                     for z in self.zero_outs]
        if self._sharding is not None:
            self._device_args = [self.jax.device_put(a, self._sharding)
                                 for a in args]
        else:
            self._device_args = [self.jax.device_put(a, self._device)
                                 for a in args]
        self.jax.block_until_ready(self._device_args)

    def run(self):
        return self._fn(*self._device_args)

    def run_numpy(self):
        out = self.jax.block_until_ready(self.run())
        n = self.n_cores
        if n == 1:
            return [{nm: np.asarray(out[i])
                     for i, nm in enumerate(self.out_names)}]
        return [{nm: np.asarray(out[i]).reshape(
                    n, *self.out_avals[i].shape)[c]
                 for i, nm in enumerate(self.out_names)}
                for c in range(n)]

    def time_exec(self, iters=8, warmup=2):
        import time
        for _ in range(warmup):
            self.jax.block_until_ready(self.run())
        ts = []
        for _ in range(iters):
            t0 = time.perf_counter()
            self.jax.block_until_ready(self.run())
            ts.append(time.perf_counter() - t0)
        return float(np.median(ts))


_CACHE = {}


def _get_runner(cfg, repeat=1):
    key = (tuple(sorted(cfg.items())), repeat)
    if key not in _CACHE:
        nc, names = build_kernel(cfg, repeat=repeat)
        _CACHE[key] = (BassRunner(nc, cfg['NC']), names)
    return _CACHE[key]


def kernel(hidden_states, position_ids, Wq, Wk, Wv, Wo):
    cfg = FULL_CFG
    d = derive(cfg)
    runner, names = _get_runner(cfg)
    in_maps = [prep_core_inputs(cfg, names, hidden_states, position_ids,
                                Wq, Wk, Wv, Wo, c)
               for c in range(cfg['NC'])]
    runner.stage(in_maps)
    res = runner.run_numpy()
    B, S, H, Q = cfg['B'], cfg['S'], cfg['H'], d['Q']
    npb = cfg['NC'] // cfg['B']
    out = np.empty((B, S, H), np.float32)
    for c in range(cfg['NC']):
        b, p = c // npb, c % npb
        out[b, p * Q:(p + 1) * Q, :] = res[c][names['out']]
    return out


# revision 19
# speedup vs baseline: 2.8801x; 2.8801x over previous
"""Llama attention (B=2, S=2048, H=4096, 32 heads / 8 KV heads) on 8
Trainium2 NeuronCores.

Sharding: query-token parallel. Core c handles batch c//4, query rows
[512*(c%4), 512*(c%4)+512). Each core computes the full K/V projection for
its batch (replicated within its 4-core batch group), attention for its
512 query rows against all 2048 keys (causality via a 0/1 mask supplied as
data), and the o_proj for its rows. No collectives; the host concatenates
the 8 per-core row slices into the full output.

All matmuls run in fp16 with fp32 PSUM accumulation. The host pre-packs
every operand into the exact SBUF tile layout (hidden-on-partitions,
128-partition tiles) so device DMAs are large contiguous reads.
"""
import sys
import os
sys.path.insert(0, '/opt/trn_rl_repo')
import numpy as np

FULL_CFG = dict(H=4096, NH=32, NKV=8, HD=128, B=2, S=2048, NC=8)


def derive(cfg):
    d = dict(cfg)
    d['hk'] = cfg['H'] // 128            # hidden 128-tiles
    d['Q'] = cfg['S'] // (cfg['NC'] // cfg['B'])   # query rows per core
    d['QC'] = d['Q'] // 128              # query 128-chunks
    d['KT'] = cfg['S'] // 128            # key 128-tiles
    d['NP'] = cfg['S'] // 512            # 512-token panels
    d['VD'] = cfg['NKV'] * 128           # total kv dim
    d['VW'] = min(512, d['VD'])          # vd panel width
    d['VDP'] = d['VD'] // d['VW']
    d['FT'] = cfg['H'] // 128            # o_proj f tiles (= hk)
    d['OW'] = min(512, cfg['H'])         # o_proj out panel width
    d['OD'] = cfg['H'] // d['OW']
    return d


def build_kernel(cfg, repeat=1):
    import concourse.bacc as bacc
    import concourse.tile as tile
    import concourse.mybir as mybir

    d = derive(cfg)
    H, S, Q, NH, NKV = cfg['H'], cfg['S'], d['Q'], cfg['NH'], cfg['NKV']
    hk, QC, KT, NP = d['hk'], d['QC'], d['KT'], d['NP']
    VW, VDP, FT, OW, OD = d['VW'], d['VDP'], d['FT'], d['OW'], d['OD']
    f16, f32 = mybir.dt.float16, mybir.dt.float32
    EXP = mybir.ActivationFunctionType.Exp
    SCALE = 1.0 / float(np.sqrt(cfg['HD']))
    EBIAS = -5.0

    PH = os.environ.get("KPHASES", "ABCDE")
    nc = bacc.Bacc(target_bir_lowering=False)
    with tile.TileContext(nc) as tc:
        with tc.tile_pool(name="dram", bufs=1, space="DRAM") as dram:
            xt_d = dram.tile([128, hk, S], f16, kind="ExternalInput")
            wq_d = dram.tile([128, NH, hk, 128], f16, kind="ExternalInput")
            wk_d = dram.tile([128, NKV, hk, 128], f16, kind="ExternalInput")
            wv_d = dram.tile([128, VDP, hk, VW], f16, kind="ExternalInput")
            wo_d = dram.tile([128, OD, FT, OW], f16, kind="ExternalInput")
            ck_d = dram.tile([128, S], f16, kind="ExternalInput")
            sk_d = dram.tile([128, S], f16, kind="ExternalInput")
            mk_d = dram.tile([128, KT, Q], f16, kind="ExternalInput")
            out_d = dram.tile([Q, H], f32, kind="ExternalOutput")
            names = dict(xt=xt_d.name, wq=wq_d.name, wk=wk_d.name,
                         wv=wv_d.name, wo=wo_d.name, ck=ck_d.name,
                         sk=sk_d.name, mk=mk_d.name, out=out_d.name)

            with tc.tile_pool(name="const", bufs=1) as cpool, \
                 tc.tile_pool(name="res", bufs=1) as res, \
                 tc.tile_pool(name="qa", bufs=NH + 1) as qa, \
                 tc.tile_pool(name="rope", bufs=1) as rope, \
                 tc.tile_pool(name="ptile", bufs=4) as ptile, \
                 tc.tile_pool(name="ps2", bufs=2, space="PSUM") as ps2, \
                 tc.tile_pool(name="ps3", bufs=3, space="PSUM") as ps3, \
                 tc.tile_pool(name="psd", bufs=1, space="PSUM") as psd:

                bias_t = cpool.tile([128, 1], f32, name="bias_t")
                nc.vector.memset(bias_t[:], EBIAS)
                ones_c = cpool.tile([128, 1], f16, name="ones_c")
                nc.vector.memset(ones_c[:], 1.0)

                ck_s = res.tile([128, S], f16, name="ck_s")
                sk_s = res.tile([128, S], f16, name="sk_s")
                nc.sync.dma_start(ck_s[:], ck_d[:])
                nc.sync.dma_start(sk_s[:], sk_d[:])

                kt_kv = [res.tile([128, S], f16, name=f"ktkv{i}")
                         for i in range(NKV)]
                v_half = [res.tile([128, KT * VW], f16, name=f"vh{i}")
                          for i in range(VDP)]

                def rope_evict(psum, out_slice, tab_lo, tab_hi):
                    # out = base*cos + shift64(base)*sin' over token cols
                    # [tab_lo, tab_hi) of the rope tables
                    n = tab_hi - tab_lo
                    base = rope.tile([128, n], f32, tag="base", name="base")
                    shf = rope.tile([128, n], f32, tag="shf", name="shf")
                    m1 = rope.tile([128, n], f32, tag="m1", name="m1")
                    nc.scalar.copy(base[:], psum[:])
                    nc.sync.dma_start(shf[0:64, :], base[64:128, :])
                    nc.sync.dma_start(shf[64:128, :], base[0:64, :])
                    nc.vector.tensor_mul(m1[:], shf[:],
                                         sk_s[:, tab_lo:tab_hi])
                    nc.vector.tensor_mul(base[:], base[:],
                                         ck_s[:, tab_lo:tab_hi])
                    nc.vector.tensor_add(out_slice, m1[:], base[:])

                for rep in range(repeat):
                    # ---- phase A: q-proj + RoPE -> qt tiles ----
                    qt_tiles = []
                    with tc.tile_pool(name=f"stA{rep}", bufs=2) as stA, \
                         tc.tile_pool(name=f"xtA{rep}", bufs=1) as xtA:
                        hq = hk // 4
                        xt0q = []
                        for qq in range(4):
                            t = xtA.tile([128, hq * Q], f16, tag=f"xq{qq}",
                                         name=f"xt0_{rep}_{qq}")
                            nc.sync.dma_start(
                                t[:].rearrange("p (a b) -> p a b", a=hq),
                                xt_d[:, qq * hq:(qq + 1) * hq, 0:Q])
                            xt0q.append(t)
                        for qd in range(NH):
                            wq_t = stA.tile([128, hk * 128], f16, tag="wq",
                                            name=f"wq{qd}")
                            nc.sync.dma_start(
                                wq_t[:].rearrange("p (a b) -> p a b", a=hk),
                                wq_d[:, qd])
                            pq = ps2.tile([128, Q], f32, tag="pp",
                                          name=f"pq{qd}")
                            for k in range(hk):
                                hq4 = hk // 4
                                src_t = xt0q[k // hq4]
                                kk = k % hq4
                                nc.tensor.matmul(
                                    pq[:], wq_t[:, k * 128:(k + 1) * 128],
                                    src_t[:, kk * Q:kk * Q + Q],
                                    start=(k == 0), stop=(k == hk - 1))
                            qt = qa.tile([128, Q], f16, tag="qa",
                                         name=f"qt{qd}")
                            rope_evict(pq[:], qt[:], 0, Q)
                            qt_tiles.append(qt)

                    # ---- phase B: V-proj (token-major) ----
                    NB = S // 256  # 256-token chunks for xt double-buffer
                    with tc.tile_pool(name=f"stB{rep}", bufs=1) as stB, \
                         tc.tile_pool(name=f"xtB{rep}", bufs=2) as xtB:
                        for vdp in (range(VDP) if 'B' in PH else []):
                            hv = hk // 4
                            wv_q = []
                            for qq in range(4):
                                t = stB.tile([128, hv * VW], f16,
                                             tag=f"wvq{qq}",
                                             name=f"wv{vdp}_{qq}")
                                nc.sync.dma_start(
                                    t[:].rearrange("p (a b) -> p a b", a=hv),
                                    wv_d[:, vdp, qq * hv:(qq + 1) * hv])
                                wv_q.append(t)
                            for pan in range(NB):
                                xt1 = xtB.tile([128, hk * 256], f16,
                                               tag="xt",
                                               name=f"xtv{vdp}_{pan}")
                                nc.sync.dma_start(
                                    xt1[:].rearrange(
                                        "p (a b) -> p a b", a=hk),
                                    xt_d[:, :, pan * 256:(pan + 1) * 256])
                                for tt in range(2):
                                    pv = ps2.tile([128, VW], f32, tag="pp",
                                                  name=f"pv{vdp}{pan}{tt}")
                                    hv4 = hk // 4
                                    for k in range(hk):
                                        nc.tensor.matmul(
                                            pv[:],
                                            xt1[:, k * 256 + tt * 128:
                                                k * 256 + tt * 128 + 128],
                                            wv_q[k // hv4][
                                                :, (k % hv4) * VW:
                                                (k % hv4) * VW + VW],
                                            start=(k == 0),
                                            stop=(k == hk - 1))
                                    col = (pan * 2 + tt) * VW
                                    nc.scalar.copy(
                                        v_half[vdp][:, col:col + VW], pv[:])

                    # ---- phase C: K-proj + RoPE, kd-pair-major so that
                    # attention heads unlock progressively ----
                    hh = hk // 2
                    with tc.tile_pool(name=f"stC{rep}", bufs=2) as stC, \
                         tc.tile_pool(name=f"xtC{rep}", bufs=2) as xtC:
                        for kdp in (range((NKV + 1) // 2)
                                    if 'C' in PH else []):
                            kds = [k for k in (2 * kdp, 2 * kdp + 1)
                                   if k < NKV]
                            wk_ts = []
                            for kd in kds:
                                wk_t = stC.tile([128, hk * 128], f16,
                                                tag="wk",
                                                name=f"wk{kdp}_{kd}")
                                nc.sync.dma_start(
                                    wk_t[:].rearrange(
                                        "p (a b) -> p a b", a=hk),
                                    wk_d[:, kd])
                                wk_ts.append(wk_t)
                            for pan in range(NP):
                                xta = xtC.tile([128, hh * 512], f16,
                                               tag="xta",
                                               name=f"xta{kdp}_{pan}")
                                xtb = xtC.tile([128, hh * 512], f16,
                                               tag="xtb",
                                               name=f"xtb{kdp}_{pan}")
                                nc.sync.dma_start(
                                    xta[:].rearrange(
                                        "p (a b) -> p a b", a=hh),
                                    xt_d[:, 0:hh,
                                         pan * 512:(pan + 1) * 512])
                                nc.sync.dma_start(
                                    xtb[:].rearrange(
                                        "p (a b) -> p a b", a=hh),
                                    xt_d[:, hh:hk,
                                         pan * 512:(pan + 1) * 512])
                                for kd, wk_t in zip(kds, wk_ts):
                                    pk = ps2.tile([128, 512], f32,
                                                  tag="pp",
                                                  name=f"pk{kdp}{pan}{kd}")
                                    for k in range(hk):
                                        src = xta if k < hh else xtb
                                        kk = k if k < hh else k - hh
                                        nc.tensor.matmul(
                                            pk[:],
                                            wk_t[:, k * 128:(k + 1) * 128],
                                            src[:, kk * 512:(kk + 1) * 512],
                                            start=(k == 0),
                                            stop=(k == hk - 1))
                                    rope_evict(
                                        pk[:],
                                        kt_kv[kd][:, pan * 512:
                                                  pan * 512 + 512],
                                        pan * 512, (pan + 1) * 512)

                    # ---- phase D: attention (+ o_proj halves inline) ----
                    def attn_head(h, mk_s):
                        kv = h // (NH // NKV)
                        ppv = ps2.tile([128, Q], f32, tag="pv",
                                       name=f"ppv{h}")
                        pden = psd.tile([1, Q], f32, tag="den",
                                        name=f"pden{h}")
                        for kt in range(KT):
                            pst = ps3.tile([128, Q], f32, tag="s",
                                           name=f"pst{h}_{kt}")
                            nc.tensor.matmul(
                                pst[:],
                                kt_kv[kv][:, kt * 128:kt * 128 + 128],
                                qt_tiles[h][:],
                                start=True, stop=True)
                            pt = ptile.tile([128, Q], f16, tag="pt",
                                            name=f"pt{h}_{kt}")
                            nc.scalar.activation(
                                pt[:], pst[:], EXP,
                                bias=bias_t[:], scale=SCALE)
                            nc.vector.tensor_mul(
                                pt[:], pt[:],
                                mk_s[:, kt * Q:(kt + 1) * Q])
                            vh = v_half[(kv * 128) // VW]
                            vcol = kt * VW + (kv * 128) % VW
                            nc.tensor.matmul(
                                ppv[:], vh[:, vcol:vcol + 128], pt[:],
                                start=(kt == 0), stop=(kt == KT - 1))
                            nc.tensor.matmul(
                                pden[:], ones_c[:], pt[:],
                                start=(kt == 0), stop=(kt == KT - 1))
                        den_r = rope.tile([1, Q], f32, tag="den_r",
                                          name=f"den_r{h}")
                        nc.vector.reciprocal(den_r[:], pden[:])
                        bc = rope.tile([128, Q], f32, tag="bc",
                                       name=f"bc{h}")
                        nc.gpsimd.partition_broadcast(bc[:], den_r[:])
                        at = qa.tile([128, Q], f16, tag="qa",
                                     name=f"at{h}")
                        nc.vector.tensor_mul(at[:], ppv[:], bc[:])
                        at_tiles.append(at)

                    def oproj_half(half, stE):
                        # out[qc,od] (+)= sum_{ft in half} attnT_ft @ WoT
                        f0, f1 = half * (FT // 2), (half + 1) * (FT // 2)
                        nf = f1 - f0
                        for od in range(OD):
                            wo_t = stE.tile([128, nf * OW], f16, tag="wo",
                                            name=f"wo{half}_{od}")
                            nc.sync.dma_start(
                                wo_t[:].rearrange("p (a b) -> p a b", a=nf),
                                wo_d[:, od, f0:f1])
                            for qc in range(QC):
                                po = ps2.tile([128, OW], f32, tag="pp",
                                              name=f"po{half}{od}{qc}")
                                for i, ft in enumerate(range(f0, f1)):
                                    nc.tensor.matmul(
                                        po[:],
                                        at_tiles[ft][:, qc * 128:
                                                     qc * 128 + 128],
                                        wo_t[:, i * OW:(i + 1) * OW],
                                        start=(i == 0),
                                        stop=(i == nf - 1))
                                stg = ptile.tile([128, OW], f32, tag="stg",
                                                 name=f"stg{half}{od}{qc}")
                                nc.scalar.copy(stg[:], po[:])
                                if half == 0:
                                    nc.sync.dma_start(
                                        out_d[qc * 128:(qc + 1) * 128,
                                              od * OW:(od + 1) * OW],
                                        stg[:])
                                else:
                                    nc.gpsimd.dma_start(
                                        out_d[qc * 128:(qc + 1) * 128,
                                              od * OW:(od + 1) * OW],
                                        stg[:],
                                        accum_op=mybir.AluOpType.add)

                    at_tiles = []
                    with tc.tile_pool(name=f"mkp{rep}", bufs=1) as mkp, \
                         tc.tile_pool(name=f"stE{rep}", bufs=2) as stE:
                        mk_s = mkp.tile([128, KT * Q], f16,
                                        name=f"mk_s{rep}")
                        nc.sync.dma_start(
                            mk_s[:].rearrange("p (a b) -> p a b", a=KT),
                            mk_d[:])
                        if 'D' in PH:
                            for h in range(NH // 2):
                                attn_head(h, mk_s)
                            if 'E' in PH:
                                oproj_half(0, stE)
                            for h in range(NH // 2, NH):
                                attn_head(h, mk_s)
                            if 'E' in PH:
                                oproj_half(1, stE)
    nc.compile()
    return nc, names


def prep_core_inputs(cfg, names, hidden, position_ids, Wq, Wk, Wv, Wo, core):
    d = derive(cfg)
    H, S, Q, NH, NKV, HD = (cfg['H'], cfg['S'], d['Q'], cfg['NH'],
                            cfg['NKV'], cfg['HD'])
    hk, KT = d['hk'], d['KT']
    VW, VDP, FT, OW, OD = d['VW'], d['VDP'], d['FT'], d['OW'], d['OD']
    npb = cfg['NC'] // cfg['B']
    b, p = core // npb, core % npb
    f16 = np.float16

    X = np.roll(np.asarray(hidden[b]), -p * Q, axis=0)      # [S, H]
    xt = np.ascontiguousarray(
        X.T.reshape(hk, 128, S).transpose(1, 0, 2)).astype(f16)

    wq = np.ascontiguousarray(
        np.asarray(Wq).reshape(NH, 128, hk, 128)
        .transpose(3, 0, 2, 1)).astype(f16)
    wk = np.ascontiguousarray(
        np.asarray(Wk).reshape(NKV, 128, hk, 128)
        .transpose(3, 0, 2, 1)).astype(f16)
    wv = np.ascontiguousarray(
        np.asarray(Wv).reshape(VDP, VW, hk, 128)
        .transpose(3, 0, 2, 1)).astype(f16)
    wo = np.ascontiguousarray(
        np.asarray(Wo).reshape(OD, OW, FT, 128)
        .transpose(3, 0, 2, 1)).astype(f16)

    pos = np.roll(np.asarray(position_ids[b]).astype(np.float64), -p * Q)
    inv_freq = 1.0 / (10000.0 ** (np.arange(0, HD, 2) / HD))
    ang = pos[:, None] * inv_freq[None, :]                  # [S, HD/2]
    ang2 = np.concatenate([ang, ang], axis=1).T             # [HD, S]
    sk = np.sin(ang2).astype(np.float32)
    ck = np.cos(ang2).astype(np.float32)
    sk[0:HD // 2, :] *= -1.0

    gq = p * Q + np.arange(Q)
    gk = (np.arange(S) + p * Q) % S
    mask = (gk[:, None] <= gq[None, :]).astype(f16)         # [S, Q]
    mk = np.ascontiguousarray(
        mask.reshape(KT, 128, Q).transpose(1, 0, 2))

    return {
        names['xt']: xt,
        names['wq']: wq, names['wk']: wk, names['wv']: wv, names['wo']: wo,
        names['ck']: ck.astype(f16), names['sk']: sk.astype(f16),
        names['mk']: mk,
    }


# ---------------------------------------------------------------------------
# PJRT runner (embedded so kernel.py is self-contained)
# ---------------------------------------------------------------------------
class BassRunner:
    def __init__(self, nc, n_cores):
        import jax
        import concourse.mybir as mybir
        from concourse.bass2jax import (
            _bass_exec_p, install_neuronx_cc_hook, partition_id_tensor)
        from jax.sharding import Mesh, PartitionSpec
        from jax.experimental.shard_map import shard_map
        self.jax = jax
        install_neuronx_cc_hook()
        self.nc = nc
        self.n_cores = n_cores

        partition_name = (nc.partition_id_tensor.name
                          if nc.partition_id_tensor else None)
        dbg_name = nc.dbg_addr.name if nc.dbg_addr is not None else None
        in_names, out_names, out_avals, zero_outs = [], [], [], []
        for alloc in nc.m.functions[0].allocations:
            if not isinstance(alloc, mybir.MemoryLocationSet):
                continue
            name = alloc.memorylocations[0].name
            if alloc.kind == "ExternalInput":
                if name not in (partition_name, dbg_name):
                    in_names.append(name)
            elif alloc.kind == "ExternalOutput":
                shape = tuple(alloc.tensor_shape)
                dtype = mybir.dt.np(alloc.dtype)
                out_names.append(name)
                out_avals.append(jax.core.ShapedArray(shape, dtype))
                zero_outs.append(np.zeros(shape, dtype))
        self.in_names, self.out_names = in_names, out_names
        self.out_avals, self.zero_outs = out_avals, zero_outs
        n_params = len(in_names)
        self.n_params = n_params

        all_in_names = list(in_names) + list(out_names)
        if dbg_name is not None:
            all_in_names.append(dbg_name)
        if partition_name is not None:
            all_in_names.append(partition_name)
        has_dbg = dbg_name is not None

        def _body(*args):
            operands = list(args)
            if has_dbg:
                operands.append(jax.numpy.zeros((1, 2), jax.numpy.uint32))
            if partition_name is not None:
                operands.append(partition_id_tensor())
            outs = _bass_exec_p.bind(
                *operands,
                out_avals=tuple(out_avals),
                in_names=tuple(all_in_names),
                out_names=tuple(out_names),
                lowering_input_output_aliases=(),
                sim_require_finite=False,
                sim_require_nnan=False,
                nc=nc,
            )
            return tuple(outs)

        devices = [d for d in jax.devices() if d.platform != 'cpu'][:n_cores]
        assert len(devices) == n_cores, \
            f"need {n_cores} devices, have {len(jax.devices())}"
        self._sharding = None
        if n_cores == 1:
            self._device = devices[0]
            self._fn = jax.jit(_body, keep_unused=True)
        else:
            mesh = Mesh(np.asarray(devices), ("core",))
            self._sharding = jax.sharding.NamedSharding(
                mesh, PartitionSpec("core"))
            in_specs = (PartitionSpec("core"),) * (n_params + len(out_names))
            out_specs = (PartitionSpec("core"),) * len(out_names)
            self._fn = jax.jit(
                shard_map(_body, mesh=mesh, in_specs=in_specs,
                          out_specs=out_specs, check_rep=False),
                keep_unused=True)
        self._device_args = None

    def stage(self, in_maps):
        assert len(in_maps) == self.n_cores
        n = self.n_cores
        if n == 1:
            args = [np.asarray(in_maps[0][nm]) for nm in self.in_names]
            args += list(self.zero_outs)
        else:
            args = [np.concatenate([np.asarray(in_maps[c][nm])
                                    for c in range(n)], axis=0)
                    for nm in self.in_names]
            args += [np.zeros((n * z.shape[0], *z.shape[1:]), z.dtype)
                     for z in self.zero_outs]
        if self._sharding is not None:
            self._device_args = [self.jax.device_put(a, self._sharding)
                                 for a in args]
        else:
            self._device_args = [self.jax.device_put(a, self._device)
                                 for a in args]
        self.jax.block_until_ready(self._device_args)

    def run(self):
        return self._fn(*self._device_args)

    def run_numpy(self):
        out = self.jax.block_until_ready(self.run())
        n = self.n_cores
        if n == 1:
            return [{nm: np.asarray(out[i])
                     for i, nm in enumerate(self.out_names)}]
        return [{nm: np.asarray(out[i]).reshape(
                    n, *self.out_avals[i].shape)[c]
                 for i, nm in enumerate(self.out_names)}
                for c in range(n)]

    def time_exec(self, iters=8, warmup=2):
        import time
        for _ in range(warmup):
            self.jax.block_until_ready(self.run())
        ts = []
        for _ in range(iters):
            t0 = time.perf_counter()
            self.jax.block_until_ready(self.run())
            ts.append(time.perf_counter() - t0)
        return float(np.median(ts))


_CACHE = {}


def _get_runner(cfg, repeat=1):
    key = (tuple(sorted(cfg.items())), repeat)
    if key not in _CACHE:
        nc, names = build_kernel(cfg, repeat=repeat)
        _CACHE[key] = (BassRunner(nc, cfg['NC']), names)
    return _CACHE[key]


def kernel(hidden_states, position_ids, Wq, Wk, Wv, Wo):
    cfg = FULL_CFG
    d = derive(cfg)
    runner, names = _get_runner(cfg)
    in_maps = [prep_core_inputs(cfg, names, hidden_states, position_ids,
                                Wq, Wk, Wv, Wo, c)
               for c in range(cfg['NC'])]
    runner.stage(in_maps)
    res = runner.run_numpy()
    B, S, H, Q = cfg['B'], cfg['S'], cfg['H'], d['Q']
    npb = cfg['NC'] // cfg['B']
    out = np.empty((B, S, H), np.float32)
    for c in range(cfg['NC']):
        b, p = c // npb, c % npb
        out[b, p * Q:(p + 1) * Q, :] = res[c][names['out']]
    return out
